# revision 29
# baseline (speedup 1.0000x reference)
"""CrossLayerTranscoder with global batch-wise top-k masking on 8 TRN2 cores.

Reference computation:
    pre = relu(x @ W_enc + b_enc)            [4096, 16384]
    keep the global top-(top_k * 4096) entries, zero the rest.

Device algorithm (single pass, dict-sharded over 8 cores), a 4-engine
pipeline per [128 cols x 512 rows] tile:
  * PE: GEMM in single-term bf16 (1 cycle/row; differential noise on
    pre_acts ~1.6e-3 rms - everything accuracy-critical is repaired on
    the host).  Transposed orientation: partition = dict col, free = row.
  * ACT: a = 4096*z + (M + 4096*b) with M = 1.5*2^23.  The f32 store
    rounds to M + q exactly (ulp(M)=1), i.e. q = round(4096*(z+b)) -
    a Round op built from the classic big-constant trick.
  * Pool: packed = (a - M) + iota/512 = q + i/512, exact in f32
    (|q| < 2^15, i < 512 -> 24 mantissa bits).  Value-major,
    index-minor packing.
  * DVE: single MAX8 per tile extracts the top-8 packed values per
    (dict col, 512-row block) - candidate value AND row index in one
    scan.  No MATCH_VALUE_LOAD / FIND_INDEX8 second pass.
  * Global merge on host:
      - decode q = floor(packed), i = (packed-q)*512; v_hat = q/4096.
      - tau_hat = k-th largest candidate.
      - 'saturated' chunks (8th candidate still >= tau_hat - DELTA) may
        hide more top-k members beyond the extracted 8: recompute those
        chunks' dot products exactly on host (~hundreds).
      - candidates within +-DELTA of tau_hat get exact recomputation
        too; exact values decide the top-k boundary, so the selected
        SET matches the reference's (a swapped element costs ~2*tau^2
        in norm^2, which is what the metric is sensitive to).
      - confident candidates (> tau_hat + DELTA) keep the quantized
        device value; its ~1.6e-3 noise is far inside the error budget.
"""

import numpy as np

P = 128
N_TOTAL = 4096
K_DIM = 768
DICT = 16384
N_CORES = 8
DICT_SH = DICT // N_CORES     # 2048
KCH = K_DIM // P              # 6
R_BLK = 512
R_BLOCKS = N_TOTAL // R_BLK   # 8
D_TILES = DICT_SH // P        # 16
CW = 8                        # top-8 per (col, 512-row block)
DGRP = 4                      # d-tiles packed per STT instruction
DELTA = 2.5e-2                # band half-width around tau_hat (~15 sigma)
MROUND = 1.5 * 2.0**23        # fp32 round-to-int magic constant
QSCALE = 4096.0               # value quantization: q = round(4096*(z+b))

_cache = {}


def _build_sparse():
    import concourse.mybir as mybir
    import concourse.tile as tile
    from concourse import bacc

    f32 = mybir.dt.float32
    bf16 = mybir.dt.bfloat16

    nc = bacc.Bacc("TRN2", target_bir_lowering=False, debug=False,
                   num_devices=N_CORES)
    # host pre-arranged layouts: xT[p, c, n] = x.T[c*128+p, n],
    # wh[p, c, n] = W[c*128+p, n]  (partition-major for 1-descriptor DMAs)
    xT = nc.dram_tensor("xT", [P, KCH * N_TOTAL], bf16, kind="ExternalInput")
    wh = nc.dram_tensor("wh", [P, KCH * DICT_SH], bf16, kind="ExternalInput")
    b = nc.dram_tensor("b", [P, D_TILES], f32, kind="ExternalInput")
    cio = nc.dram_tensor("cio", [P, R_BLK], f32, kind="ExternalInput")
    cval = nc.dram_tensor("cval", [R_BLOCKS * P, D_TILES * CW], f32,
                          kind="ExternalOutput")

    with tile.TileContext(nc) as tc:
        with (
            tc.tile_pool(name="resident", bufs=1) as rpool,
            tc.tile_pool(name="xstream", bufs=3) as xpool,
            tc.tile_pool(name="act", bufs=3) as apool,
            tc.tile_pool(name="pack", bufs=3) as ppool,
            tc.tile_pool(name="cand", bufs=2) as cpool,
            tc.tile_pool(name="psum", bufs=7, space="PSUM") as psum_pool,
            tc.tile_pool(name="warm", bufs=1, space="PSUM") as warm_pool,
        ):
            wh_sb = rpool.tile([P, KCH, DICT_SH], bf16)
            b_sb = rpool.tile([P, D_TILES], f32)
            io_sb = rpool.tile([P, DGRP, R_BLK], f32)

            xT_r = xT.ap().rearrange("p (c rb rr) -> p c rb rr",
                                     c=KCH, rr=R_BLK)
            wh_r = wh.ap().rearrange("p (c n) -> p c n", c=KCH)
            cval_r = cval.ap().rearrange("(rb p) w -> p rb w", p=P)

            # priority-ordered head: the first LDW needs W(k=0, cols 0:128)
            # and the first MM additionally x(r0, k=0); later W arrives in a
            # ramp that beats the PE's 128-cols-per-1.28us consumption pace
            # HAM pre-warm: ~10 dummy matmuls on never-written scratch tiles
            # run during the input-DMA wait, flipping the PE clock gate to
            # 8/8 (2.4 GHz) before the first real matmul issues
            warm_w = rpool.tile([P, P], bf16)
            warm_x = rpool.tile([P, R_BLK], bf16)
            nc.gpsimd.memset(warm_w[:], 0.0)
            nc.gpsimd.memset(warm_x[:], 0.0)
            wps = warm_pool.tile([P, R_BLK], mybir.dt.float32)
            for _ in range(8):
                nc.tensor.matmul(wps[:], warm_w[:], warm_x[:],
                                 start=True, stop=True)

            # first x block split across 4 engines' DMA queues so the
            # transfers run concurrently (per-queue HBM bandwidth is the
            # head constraint, ~100 GB/s)
            x0 = xpool.tile([P, KCH, R_BLK], bf16, tag="xh")
            nc.sync.dma_start(wh_sb[:, 0, 0:128], wh_r[:, 0, 0:128])
            nc.scalar.dma_start(x0[:, 0:2], xT_r[:, 0:2, 0])
            nc.gpsimd.dma_start(x0[:, 2:4], xT_r[:, 2:4, 0])
            nc.sync.dma_start(x0[:, 4:6], xT_r[:, 4:6, 0])
            nc.sync.dma_start(wh_sb[:, 1:, 0:128], wh_r[:, 1:, 0:128])
            nc.sync.dma_start(wh_sb[:, :, 128:256], wh_r[:, :, 128:256])
            nc.sync.dma_start(b_sb[:], b.ap())
            nc.sync.dma_start(wh_sb[:, :, 256:512], wh_r[:, :, 256:512])
            for g in range(DGRP):
                nc.sync.dma_start(io_sb[:, g], cio.ap())
            nc.sync.dma_start(wh_sb[:, :, 512:1024], wh_r[:, :, 512:1024])
            nc.sync.dma_start(wh_sb[:, :, 1024:2048], wh_r[:, :, 1024:2048])

            for r in range(R_BLOCKS):
                if r == 0:
                    xh_t = x0
                else:
                    xh_t = xpool.tile([P, KCH, R_BLK], bf16, tag="xh")
                    nc.sync.dma_start(xh_t[:], xT_r[:, :, r])
                cvb = cpool.tile([P, D_TILES, CW], f32, tag="cv")
                for dg in range(D_TILES // DGRP):
                    # the kernel's very last group drains per-tile so the
                    # post-GEMM pipeline tail is one tile, not one group
                    fine = (r == R_BLOCKS - 1 and dg == D_TILES // DGRP - 1)
                    a_sb = apool.tile([P, DGRP, R_BLK], f32, tag="a")
                    for dd in range(DGRP):
                        d = dg * DGRP + dd
                        ps = psum_pool.tile([P, R_BLK], mybir.dt.float32)
                        dsl = slice(d * P, (d + 1) * P)
                        for k in range(KCH):
                            nc.tensor.matmul(
                                ps[:], wh_sb[:, k, dsl], xh_t[:, k, :],
                                start=(k == 0), stop=(k == KCH - 1))
                        nc.scalar.activation(
                            a_sb[:, dd], ps[:],
                            mybir.ActivationFunctionType.Identity,
                            bias=b_sb[:, d:d + 1], scale=QSCALE)
                        if fine:
                            pk1 = ppool.tile([P, R_BLK], f32, tag="pk1")
                            nc.vector.scalar_tensor_tensor(
                                pk1[:], a_sb[:, dd], MROUND, io_sb[:, 0],
                                op0=mybir.AluOpType.subtract,
                                op1=mybir.AluOpType.add)
                            nc.vector.max(cvb[:, d], pk1[:])
                    if not fine:
                        pk = ppool.tile([P, DGRP, R_BLK], f32, tag="pk")
                        nc.vector.scalar_tensor_tensor(
                            pk[:], a_sb[:], MROUND, io_sb[:],
                            op0=mybir.AluOpType.subtract,
                            op1=mybir.AluOpType.add)
                        for dd in range(DGRP):
                            nc.vector.max(cvb[:, dg * DGRP + dd], pk[:, dd])
                nc.sync.dma_start(cval_r[:, r], cvb[:])
    nc.compile()
    return nc


def _get_kernel():
    if "k" not in _cache:
        _cache["k"] = _build_sparse()
    return _cache["k"]


def kernel(x, W_enc, b_enc, top_k):
    import ml_dtypes
    from concourse.bass_utils import run_bass_kernel_spmd

    x = np.ascontiguousarray(np.asarray(x), np.float32)
    W_enc = np.ascontiguousarray(np.asarray(W_enc), np.float32)
    b_enc = np.ascontiguousarray(np.asarray(b_enc), np.float32).ravel()
    top_k = int(np.asarray(top_k))
    k_tot = top_k * x.shape[0]
    out = np.zeros((N_TOTAL, DICT), np.float32)
    if k_tot <= 0:
        return out

    nc = _get_kernel()

    # partition-major host layouts: [p, c, n]
    xTp = np.ascontiguousarray(
        x.T.astype(ml_dtypes.bfloat16).reshape(KCH, P, N_TOTAL)
        .transpose(1, 0, 2).reshape(P, KCH * N_TOTAL))
    W16 = W_enc.astype(ml_dtypes.bfloat16)
    iot = np.tile((np.arange(R_BLK, dtype=np.float32) / 512.0)[None, :],
                  (P, 1))
    ins = []
    for c in range(N_CORES):
        sl = slice(c * DICT_SH, (c + 1) * DICT_SH)
        whp = np.ascontiguousarray(
            W16[:, sl].reshape(KCH, P, DICT_SH)
            .transpose(1, 0, 2).reshape(P, KCH * DICT_SH))
        bsh = (MROUND + QSCALE *
               np.ascontiguousarray(b_enc[sl]).reshape(D_TILES, P).T
               ).astype(np.float32).copy()
        ins.append({"xT": xTp, "wh": whp, "b": bsh, "cio": iot})

    try:
        res = run_bass_kernel_spmd(nc, ins, core_ids=list(range(N_CORES)))
    except Exception:
        # transient device errors (e.g. NRT_EXEC_UNIT_UNRECOVERABLE) recover
        # on re-execution; one retry
        res = run_bass_kernel_spmd(nc, ins, core_ids=list(range(N_CORES)))

    # ---- global merge (host) ----
    # flat layout: [core, rb, p, d, slot];  col = c*2048 + d*128 + p,
    # row = rb*512 + i;  packed = q + i/512, q = round(4096*(z+b))
    vals = np.stack([res.results[c]["cval"] for c in range(N_CORES)])
    packed = vals.ravel().astype(np.float64)
    q = np.floor(packed)
    ii = np.rint((packed - q) * 512.0).astype(np.int64)
    vb = (q / QSCALE).astype(np.float64)

    n_flat = packed.size
    f = np.arange(n_flat, dtype=np.int64)
    c_, rem = np.divmod(f, R_BLOCKS * P * D_TILES * CW)
    rb, rem = np.divmod(rem, P * D_TILES * CW)
    p, rem = np.divmod(rem, D_TILES * CW)
    d, slot = np.divmod(rem, CW)
    col = (c_ * DICT_SH + d * P + p).astype(np.int64)
    row = rb * R_BLK + ii

    k_eff = min(k_tot, n_flat)
    tau_hat = float(np.partition(vb, -k_eff)[-k_eff])

    if tau_hat <= DELTA:
        # degenerate regime (k >= positive count): values near zero,
        # approximate selection is fine
        keep = vb > 0
        order = np.argsort(-vb[keep])[:k_tot]
        out[row[keep][order], col[keep][order]] = vb[keep][order]
        return out

    # chunk = (core, rb, p, d) <-> flat // CW; slot 7 is the chunk's 8th
    # (smallest extracted) value: if it is still near/above the threshold
    # the chunk may hide more top-k members beyond the extracted 8.
    v8 = vb[slot == 7]
    sat_chunk = np.flatnonzero(v8 >= tau_hat - DELTA)   # chunk ids
    chunk_id = f // CW
    in_sat = np.isin(chunk_id, sat_chunk)

    conf = (vb > tau_hat + DELTA) & ~in_sat
    band = (vb >= tau_hat - DELTA) & (vb <= tau_hat + DELTA) & ~in_sat

    # exact recompute pool: all rows of saturated chunks + band candidates
    er_list = [row[band]]
    ec_list = [col[band]]
    if sat_chunk.size:
        sc_, srem = np.divmod(sat_chunk, R_BLOCKS * P * D_TILES)
        srb, srem = np.divmod(srem, P * D_TILES)
        sp, sd = np.divmod(srem, D_TILES)
        scol = sc_ * DICT_SH + sd * P + sp
        er_list.append(
            (srb[:, None] * R_BLK + np.arange(R_BLK)[None, :]).ravel())
        ec_list.append(np.repeat(scol, R_BLK))
    er = np.concatenate(er_list)
    ec = np.concatenate(ec_list)
    # dedupe exact positions
    epos = er * DICT + ec
    epos, uq = np.unique(epos, return_index=True)
    er, ec = er[uq], ec[uq]

    ev = np.empty(er.size, np.float64)
    CH = 65536
    for i in range(0, er.size, CH):
        s = slice(i, i + CH)
        ev[s] = np.einsum(
            "ij,ij->i",
            x[er[s]].astype(np.float64),
            W_enc[:, ec[s]].T.astype(np.float64),
            optimize=True) + b_enc[ec[s]]

    # confident candidates are all truly in the top-k (their true value is
    # > tau_hat + DELTA - noise > tau); duplicated positions carry
    # identical values, so plain assignment is safe
    out[row[conf], col[conf]] = vb[conf].astype(np.float32)
    n_conf = np.unique(row[conf] * DICT + col[conf]).size

    need = k_tot - n_conf
    if need > 0:
        # exact values decide the boundary; ties -> lowest flat index,
        # matching jax.lax.top_k
        order = np.lexsort((epos, -ev.astype(np.float64)))
        kept = order[:need]
        out[er[kept], ec[kept]] = np.maximum(ev[kept], 0)
    return out


# revision 33
# speedup vs baseline: 1.0410x; 1.0410x over previous
"""CrossLayerTranscoder with global batch-wise top-k masking on 8 TRN2 cores.

Reference computation:
    pre = relu(x @ W_enc + b_enc)            [4096, 16384]
    keep the global top-(top_k * 4096) entries, zero the rest.

Device algorithm (single pass, dict-sharded over 8 cores), a 4-engine
pipeline per [128 cols x 512 rows] tile:
  * PE: GEMM in single-term bf16 (1 cycle/row; differential noise on
    pre_acts ~1.6e-3 rms - everything accuracy-critical is repaired on
    the host).  Transposed orientation: partition = dict col, free = row.
  * ACT: a = 4096*z + (M + 4096*b) with M = 1.5*2^23.  The f32 store
    rounds to M + q exactly (ulp(M)=1), i.e. q = round(4096*(z+b)) -
    a Round op built from the classic big-constant trick.
  * Pool: packed = (a - M) + iota/512 = q + i/512, exact in f32
    (|q| < 2^15, i < 512 -> 24 mantissa bits).  Value-major,
    index-minor packing.
  * DVE: single MAX8 per tile extracts the top-8 packed values per
    (dict col, 512-row block) - candidate value AND row index in one
    scan.  No MATCH_VALUE_LOAD / FIND_INDEX8 second pass.
  * Global merge on host:
      - decode q = floor(packed), i = (packed-q)*512; v_hat = q/4096.
      - tau_hat = k-th largest candidate.
      - 'saturated' chunks (8th candidate still >= tau_hat - DELTA) may
        hide more top-k members beyond the extracted 8: recompute those
        chunks' dot products exactly on host (~hundreds).
      - candidates within +-DELTA of tau_hat get exact recomputation
        too; exact values decide the top-k boundary, so the selected
        SET matches the reference's (a swapped element costs ~2*tau^2
        in norm^2, which is what the metric is sensitive to).
      - confident candidates (> tau_hat + DELTA) keep the quantized
        device value; its ~1.6e-3 noise is far inside the error budget.
"""

import numpy as np

P = 128
N_TOTAL = 4096
K_DIM = 768
DICT = 16384
N_CORES = 8
DICT_SH = DICT // N_CORES     # 2048
KCH = K_DIM // P              # 6
R_BLK = 512
R_BLOCKS = N_TOTAL // R_BLK   # 8
D_TILES = DICT_SH // P        # 16
CW = 8                        # top-8 per (col, 512-row block)
DGRP = 4                      # d-tiles packed per STT instruction
DELTA = 2.5e-2                # band half-width around tau_hat (~15 sigma)
MROUND = 1.5 * 2.0**23        # fp32 round-to-int magic constant
QSCALE = 4096.0               # value quantization: q = round(4096*(z+b))

_cache = {}


def _build_sparse():
    import concourse.mybir as mybir
    import concourse.tile as tile
    from concourse import bacc

    f32 = mybir.dt.float32
    bf16 = mybir.dt.bfloat16

    nc = bacc.Bacc("TRN2", target_bir_lowering=False, debug=False,
                   num_devices=N_CORES)
    # host pre-arranged layouts: xT[p, c, n] = x.T[c*128+p, n],
    # wh[p, c, n] = W[c*128+p, n]  (partition-major for 1-descriptor DMAs)
    xT = nc.dram_tensor("xT", [P, KCH * N_TOTAL], bf16, kind="ExternalInput")
    wh = nc.dram_tensor("wh", [P, KCH * DICT_SH], bf16, kind="ExternalInput")
    b = nc.dram_tensor("b", [P, D_TILES], f32, kind="ExternalInput")
    cio = nc.dram_tensor("cio", [P, R_BLK], f32, kind="ExternalInput")
    cval = nc.dram_tensor("cval", [R_BLOCKS * P, D_TILES * CW], f32,
                          kind="ExternalOutput")

    with tile.TileContext(nc) as tc:
        with (
            tc.tile_pool(name="resident", bufs=1) as rpool,
            tc.tile_pool(name="xstream", bufs=3) as xpool,
            tc.tile_pool(name="act", bufs=8) as apool,
            tc.tile_pool(name="pack", bufs=8) as ppool,
            tc.tile_pool(name="cand", bufs=2) as cpool,
            tc.tile_pool(name="psum", bufs=8, space="PSUM") as psum_pool,
        ):
            wh_sb = rpool.tile([P, KCH, DICT_SH], bf16)
            b_sb = rpool.tile([P, D_TILES], f32)
            io_sb = rpool.tile([P, R_BLK], f32)

            xT_r = xT.ap().rearrange("p (c rb rr) -> p c rb rr",
                                     c=KCH, rr=R_BLK)
            wh_r = wh.ap().rearrange("p (c n) -> p c n", c=KCH)
            cval_r = cval.ap().rearrange("(rb p) w -> p rb w", p=P)

            # priority-ordered head: the first LDW needs W(k=0, cols 0:128)
            # and the first MM additionally x(r0); later W arrives in a
            # ramp that beats the PE's 128-cols-per-1.28us consumption pace
            x0 = xpool.tile([P, KCH, R_BLK], bf16, tag="xh")
            nc.sync.dma_start(wh_sb[:, 0, 0:128], wh_r[:, 0, 0:128])
            nc.sync.dma_start(x0[:], xT_r[:, :, 0])
            nc.sync.dma_start(wh_sb[:, 1:, 0:128], wh_r[:, 1:, 0:128])
            nc.sync.dma_start(b_sb[:], b.ap())
            nc.sync.dma_start(io_sb[:], cio.ap())
            edges = [128, 256, 512, 1024, 2048]
            for q0, q1 in zip(edges[:-1], edges[1:]):
                nc.sync.dma_start(wh_sb[:, :, q0:q1], wh_r[:, :, q0:q1])

            for r in range(R_BLOCKS):
                if r == 0:
                    xh_t = x0
                else:
                    xh_t = xpool.tile([P, KCH, R_BLK], bf16, tag="xh")
                    nc.sync.dma_start(xh_t[:], xT_r[:, :, r])
                cvb = cpool.tile([P, D_TILES, CW], f32, tag="cv")
                for d in range(D_TILES):
                    ps = psum_pool.tile([P, R_BLK], mybir.dt.float32)
                    dsl = slice(d * P, (d + 1) * P)
                    for k in range(KCH):
                        nc.tensor.matmul(
                            ps[:], wh_sb[:, k, dsl], xh_t[:, k, :],
                            start=(k == 0), stop=(k == KCH - 1))
                    a_sb = apool.tile([P, R_BLK], f32, tag="a")
                    nc.scalar.activation(
                        a_sb[:], ps[:],
                        mybir.ActivationFunctionType.Identity,
                        bias=b_sb[:, d:d + 1], scale=QSCALE)
                    pk = ppool.tile([P, R_BLK], f32, tag="pk")
                    nc.vector.scalar_tensor_tensor(
                        pk[:], a_sb[:], MROUND, io_sb[:],
                        op0=mybir.AluOpType.subtract,
                        op1=mybir.AluOpType.add)
                    nc.vector.max(cvb[:, d], pk[:])
                nc.sync.dma_start(cval_r[:, r], cvb[:])
    nc.compile()
    return nc


def _get_kernel():
    if "k" not in _cache:
        _cache["k"] = _build_sparse()
    return _cache["k"]


def kernel(x, W_enc, b_enc, top_k):
    import ml_dtypes
    from concourse.bass_utils import run_bass_kernel_spmd

    x = np.ascontiguousarray(np.asarray(x), np.float32)
    W_enc = np.ascontiguousarray(np.asarray(W_enc), np.float32)
    b_enc = np.ascontiguousarray(np.asarray(b_enc), np.float32).ravel()
    top_k = int(np.asarray(top_k))
    k_tot = top_k * x.shape[0]
    out = np.zeros((N_TOTAL, DICT), np.float32)
    if k_tot <= 0:
        return out

    nc = _get_kernel()

    # partition-major host layouts: [p, c, n]
    xTp = np.ascontiguousarray(
        x.T.astype(ml_dtypes.bfloat16).reshape(KCH, P, N_TOTAL)
        .transpose(1, 0, 2).reshape(P, KCH * N_TOTAL))
    W16 = W_enc.astype(ml_dtypes.bfloat16)
    iot = np.tile((np.arange(R_BLK, dtype=np.float32) / 512.0)[None, :],
                  (P, 1))
    ins = []
    for c in range(N_CORES):
        sl = slice(c * DICT_SH, (c + 1) * DICT_SH)
        whp = np.ascontiguousarray(
            W16[:, sl].reshape(KCH, P, DICT_SH)
            .transpose(1, 0, 2).reshape(P, KCH * DICT_SH))
        bsh = (MROUND + QSCALE *
               np.ascontiguousarray(b_enc[sl]).reshape(D_TILES, P).T
               ).astype(np.float32).copy()
        ins.append({"xT": xTp, "wh": whp, "b": bsh, "cio": iot})

    try:
        res = run_bass_kernel_spmd(nc, ins, core_ids=list(range(N_CORES)))
    except Exception:
        # transient device errors (e.g. NRT_EXEC_UNIT_UNRECOVERABLE) recover
        # on re-execution; one retry
        res = run_bass_kernel_spmd(nc, ins, core_ids=list(range(N_CORES)))

    # ---- global merge (host) ----
    # flat layout: [core, rb, p, d, slot];  col = c*2048 + d*128 + p,
    # row = rb*512 + i;  packed = q + i/512, q = round(4096*(z+b))
    vals = np.stack([res.results[c]["cval"] for c in range(N_CORES)])
    packed = vals.ravel().astype(np.float64)
    q = np.floor(packed)
    ii = np.rint((packed - q) * 512.0).astype(np.int64)
    vb = (q / QSCALE).astype(np.float64)

    n_flat = packed.size
    f = np.arange(n_flat, dtype=np.int64)
    c_, rem = np.divmod(f, R_BLOCKS * P * D_TILES * CW)
    rb, rem = np.divmod(rem, P * D_TILES * CW)
    p, rem = np.divmod(rem, D_TILES * CW)
    d, slot = np.divmod(rem, CW)
    col = (c_ * DICT_SH + d * P + p).astype(np.int64)
    row = rb * R_BLK + ii

    k_eff = min(k_tot, n_flat)
    tau_hat = float(np.partition(vb, -k_eff)[-k_eff])

    if tau_hat <= DELTA:
        # degenerate regime (k >= positive count): values near zero,
        # approximate selection is fine
        keep = vb > 0
        order = np.argsort(-vb[keep])[:k_tot]
        out[row[keep][order], col[keep][order]] = vb[keep][order]
        return out

    # chunk = (core, rb, p, d) <-> flat // CW; slot 7 is the chunk's 8th
    # (smallest extracted) value: if it is still near/above the threshold
    # the chunk may hide more top-k members beyond the extracted 8.
    v8 = vb[slot == 7]
    sat_chunk = np.flatnonzero(v8 >= tau_hat - DELTA)   # chunk ids
    chunk_id = f // CW
    in_sat = np.isin(chunk_id, sat_chunk)

    conf = (vb > tau_hat + DELTA) & ~in_sat
    band = (vb >= tau_hat - DELTA) & (vb <= tau_hat + DELTA) & ~in_sat

    # exact recompute pool: all rows of saturated chunks + band candidates
    er_list = [row[band]]
    ec_list = [col[band]]
    if sat_chunk.size:
        sc_, srem = np.divmod(sat_chunk, R_BLOCKS * P * D_TILES)
        srb, srem = np.divmod(srem, P * D_TILES)
        sp, sd = np.divmod(srem, D_TILES)
        scol = sc_ * DICT_SH + sd * P + sp
        er_list.append(
            (srb[:, None] * R_BLK + np.arange(R_BLK)[None, :]).ravel())
        ec_list.append(np.repeat(scol, R_BLK))
    er = np.concatenate(er_list)
    ec = np.concatenate(ec_list)
    # dedupe exact positions
    epos = er * DICT + ec
    epos, uq = np.unique(epos, return_index=True)
    er, ec = er[uq], ec[uq]

    ev = np.empty(er.size, np.float64)
    CH = 65536
    for i in range(0, er.size, CH):
        s = slice(i, i + CH)
        ev[s] = np.einsum(
            "ij,ij->i",
            x[er[s]].astype(np.float64),
            W_enc[:, ec[s]].T.astype(np.float64),
            optimize=True) + b_enc[ec[s]]

    # confident candidates are all truly in the top-k (their true value is
    # > tau_hat + DELTA - noise > tau); duplicated positions carry
    # identical values, so plain assignment is safe
    out[row[conf], col[conf]] = vb[conf].astype(np.float32)
    n_conf = np.unique(row[conf] * DICT + col[conf]).size

    need = k_tot - n_conf
    if need > 0:
        # exact values decide the boundary; ties -> lowest flat index,
        # matching jax.lax.top_k
        order = np.lexsort((epos, -ev.astype(np.float64)))
        kept = order[:need]
        out[er[kept], ec[kept]] = np.maximum(ev[kept], 0)
    return out
